# revision 11
# baseline (speedup 1.0000x reference)
"""Trainium2 Bass kernel for adjacency-masked multi-head attention.

Shapes: x[4,2048,128], A[2048,2048] 0/1, Wq[128,128], Wkv[256,128],
Wp[128,128], bp[128]; out = masked-softmax attention + residual.

Sharding: 8 cores = (batch b 0..3) x (query half 0..1); K/V computed per-core
over the full 2048 keys; A replicated (sliced per query half).

The kernel is PSUM-exit bound: every score element must leave PSUM through
ScalarE (ACT) or VectorE (DVE) at 1 elem/cycle/lane. Four per-tile paths
balance the engines, all masking off a single tensor MB in {0, 8192} bf16:
- 'D': one fused DVE op (affine_then_add) writes Schraudolph-exp bf16 bit
  patterns through an int16 view: z = round(s*2^7/ln2 + 8058.4 + MB).
  Masked z ~ +8058 (positive; small negative int16 = bf16 NaN patterns).
- 'P': PE adds MB into the score PSUM (identity-stationary matmuls riding
  spare TensorE cycles), then ACT does a pure exp with bias=-8192:
  exp(s + MB - 8192) -> exp(s) unmasked, 0 masked. No per-element mask op.
- 'A'/'G': ACT exp, then post-mask min(p, MB) on DVE / GPSIMD (p <= 1.74
  since |logit| < 0.6, so min with {0, 8192} is exact masking).
Softmax denominators ride the PV matmul via all-ones stationary columns;
reciprocal via the fast approx custom op; the residual (+x) rides the output
projection as an extra identity matmul; epilogue is emitted in deferred
slices so no engine serializes on its chain.
"""

import contextlib

B, N, C, H, HD = 4, 2048, 128, 4, 32
NQ = 1024
SCALE = HD ** -0.5
KB = N // 128
QC = NQ // 512

SCHRAUD_A = 184.66503      # 2^7 / ln 2
SCHRAUD_B = 16250.4 - 8192.0  # 127*2^7 - calibration - 8192 (MB adds it back)

# per-(kb, hp) tile path assignment, same for both qc chunks.
# 'D' = DVE fused Schraudolph; 'P' = PE mask-add + pure ACT exp;
# 'A'/'G' = ACT exp with min-mask on DVE/GPSIMD.
QUOTA32 = {"P": 13, "D": 13, "G": 4, "A": 2}


def _build_pattern(quota, n=32):
    acc = {t: 0.0 for t in quota}
    out = []
    for _ in range(n):
        for t in acc:
            acc[t] += quota[t] / n
        t = max(sorted(acc), key=lambda k: acc[k])
        acc[t] -= 1.0
        out.append(t)
    return out


PATTERN = _build_pattern(QUOTA32)


def tile_path(kb, hp):
    return PATTERN[2 * kb + hp]


# kb blocks whose mask goes through GPSIMD (Pool TT supports mult, not min):
# those get a dedicated {0,1} mask slice for a plain multiply.
G_KBS = sorted({kb for kb in range(KB) for hp in range(2)
                if tile_path(kb, hp) == "G"})


def _build(rep=1):
    import concourse.bacc as bacc
    import concourse.mybir as mybir
    import concourse.tile as tile
    from concourse.tile_rust import add_dep_helper

    F32 = mybir.dt.float32
    BF16 = mybir.dt.bfloat16
    I16 = mybir.dt.int16
    EXP = mybir.ActivationFunctionType.Exp
    MULT = mybir.AluOpType.mult
    MIN = mybir.AluOpType.min

    nc = bacc.Bacc("TRN2", target_bir_lowering=False, debug=False)

    xT = nc.dram_tensor("xT", [C, N], BF16, kind="ExternalInput")
    xrT = nc.dram_tensor("xrT", [C, NQ], BF16, kind="ExternalInput")
    MB = nc.dram_tensor("MB", [N, NQ], BF16, kind="ExternalInput")
    AB01 = (nc.dram_tensor("AB01", [len(G_KBS) * 128, NQ], BF16,
                           kind="ExternalInput") if G_KBS else None)
    WqT = nc.dram_tensor("WqT", [C, C], BF16, kind="ExternalInput")
    WkT = nc.dram_tensor("WkT", [C, C], BF16, kind="ExternalInput")
    WvT = nc.dram_tensor("WvT", [C, C], BF16, kind="ExternalInput")
    SELB = nc.dram_tensor("SELB", [C, C], BF16, kind="ExternalInput")
    I128 = nc.dram_tensor("I128", [C, C], BF16, kind="ExternalInput")
    WpT0 = nc.dram_tensor("WpT0", [C, C], BF16, kind="ExternalInput")
    WpT1 = nc.dram_tensor("WpT1", [C, C], BF16, kind="ExternalInput")
    bpT = nc.dram_tensor("bpT", [C, 1], F32, kind="ExternalInput")
    outT = nc.dram_tensor("outT", [C, NQ], F32, kind="ExternalOutput")

    with tile.TileContext(nc) as tc:
        loop_cm = tc.For_i(0, rep) if rep > 1 else contextlib.nullcontext()
        with loop_cm:
            with (
                tc.tile_pool(name="const", bufs=1) as cpool,
                tc.tile_pool(name="data", bufs=1) as dpool,
            ):
                w_q = cpool.tile([C, C], BF16, name="w_q")
                w_k = cpool.tile([C, C], BF16, name="w_k")
                w_v = cpool.tile([C, C], BF16, name="w_v")
                selb = cpool.tile([C, C], BF16, name="selb")
                i128 = cpool.tile([C, C], BF16, name="i128")
                w_p0 = cpool.tile([C, C], BF16, name="w_p0")
                w_p1 = cpool.tile([C, C], BF16, name="w_p1")
                bp_sb = cpool.tile([C, 1], F32, name="bp_sb")
                warm = cpool.tile([C, 1], F32, name="warm")
                nc.gpsimd.memset(warm[:], 0.0)
                nc.scalar.activation(warm[:], warm[:], EXP)
                nbias = cpool.tile([C, 1], F32, name="nbias")
                nc.gpsimd.memset(nbias[:], -8192.0)

                xr_sb = dpool.tile([C, NQ], BF16, name="xr_sb")
                # compute-critical tensors first (xT in chunks so projections
                # can start early), then per-kb mask tiles in consumption order
                nc.sync.dma_start(w_k[:], WkT[:])
                nc.sync.dma_start(w_q[:], WqT[:])
                xt_ck = []
                for ch in range(4):
                    t = dpool.tile([C, 512], BF16, name=f"xt{ch}")
                    eng = nc.scalar if ch < 2 else nc.sync
                    eng.dma_start(t[:], xT[:, ch * 512:(ch + 1) * 512])
                    xt_ck.append(t)
                nc.sync.dma_start(w_v[:], WvT[:])
                nc.sync.dma_start(selb[:], SELB[:])
                nc.sync.dma_start(i128[:], I128[:])
                nc.sync.dma_start(w_p0[:], WpT0[:])
                nc.sync.dma_start(w_p1[:], WpT1[:])
                nc.sync.dma_start(bp_sb[:], bpT[:])
                nc.sync.dma_start(xr_sb[:], xrT[:])
                mb_sb = []
                for kb in range(KB):
                    t = dpool.tile([128, NQ], BF16, name=f"mb{kb}")
                    nc.sync.dma_start(t[:], MB[kb * 128:(kb + 1) * 128, :])
                    mb_sb.append(t)
                ab_sb = {}
                for gi, kb in enumerate(G_KBS):
                    t = dpool.tile([128, NQ], BF16, name=f"ab{kb}")
                    nc.sync.dma_start(t[:], AB01[gi * 128:(gi + 1) * 128, :])
                    ab_sb[kb] = t

                kT_sb = dpool.tile([C, N], BF16, name="kT_sb")
                qT_sb = dpool.tile([C, NQ], BF16, name="qT_sb")
                vaug_sb = dpool.tile([128, KB * H * 64], BF16, name="vaug_sb")
                nc.gpsimd.memset(vaug_sb[:], 1.0)

                with tc.tile_pool(name="pjps", bufs=2, space="PSUM") as pjps:
                    # k-chunk0 and q-chunk0 first: scores(kb 0-3) need only
                    # these two, so the attention loop starts ~2us earlier
                    for kind, ch in (("k", 0), ("q", 0), ("k", 1), ("k", 2),
                                     ("k", 3), ("q", 1)):
                        ps = pjps.tile([C, 512], F32, name=f"p{kind}{ch}", tag="pj")
                        if kind == "k":
                            nc.tensor.matmul(ps[:], w_k[:], xt_ck[ch][:])
                            nc.scalar.copy(kT_sb[:, ch * 512:(ch + 1) * 512], ps[:])
                        else:
                            nc.tensor.matmul(ps[:], w_q[:], xt_ck[ch][:])
                            nc.scalar.copy(qT_sb[:, ch * 512:(ch + 1) * 512], ps[:])
                    for kb in range(KB):
                        ps = pjps.tile([128, C], F32, name=f"pv{kb}", tag="pj")
                        nc.tensor.matmul(
                            ps[:],
                            xt_ck[kb // 4][:, (kb % 4) * 128:(kb % 4 + 1) * 128],
                            w_v[:],
                        )
                        dst = vaug_sb[:, kb * 256:(kb + 1) * 256].rearrange(
                            "p (h x) -> p h x", x=64
                        )[:, :, 0:32]
                        src = ps[:].rearrange("p (h d) -> p h d", d=32)
                        nc.vector.tensor_copy(dst, src)

                with (
                    tc.tile_pool(name="sps", bufs=3, space="PSUM") as sps,
                    tc.tile_pool(name="accps", bufs=1, space="PSUM") as accps,
                    tc.tile_pool(name="ppool", bufs=10) as ppool,
                    tc.tile_pool(name="epool", bufs=2) as epool,
                ):
                    # epilogue work is emitted one piece per kb of the NEXT
                    # qc chunk so the in-order engines never serialize on the
                    # recip -> bc -> asc -> Wp chain
                    deferred = []

                    def run_deferred(k=1):
                        for _ in range(k):
                            if deferred:
                                deferred.pop(0)()

                    for qc in range(QC):
                        qs = slice(qc * 512, (qc + 1) * 512)
                        acc_ps = accps.tile([128, 1024], F32, name=f"acc{qc}", tag="acc")
                        last_score_mm = [None]

                        def emit_pv_one(kb, hp, p_sb, hh, acc_ps=acc_ps, lsm=last_score_mm):
                            if True:
                                h = hp * 2 + hh
                                m, b = h % 2, h // 2
                                mm = nc.tensor.matmul(
                                    acc_ps[64 * m:64 * (m + 1), b * 512:(b + 1) * 512],
                                    vaug_sb[:, kb * 256 + h * 64:kb * 256 + (h + 1) * 64],
                                    p_sb[:, hh * 512:(hh + 1) * 512],
                                    start=(kb == 0),
                                    stop=(kb == KB - 1),
                                    tile_position=(0, 64 * m),
                                )
                                if lsm[0] is not None:
                                    add_dep_helper(
                                        mm.ins, lsm[0], sync=False,
                                        reason="sw-pipeline PE order",
                                    )

                        def emit_pv(kb, hp, p_sb):
                            for hh in range(2):
                                emit_pv_one(kb, hp, p_sb, hh)

                        pending = []

                        def drain_pv():
                            # keep PV 2-3 kb blocks behind the scores so the
                            # in-order PE never waits on exp/mask completion
                            while len(pending) > 6:
                                (kb0, hp0, p0) = pending.pop(0)
                                (kb1, hp1, p1) = pending.pop(0)
                                for (kbx, hpx, px, hh) in (
                                    (kb0, hp0, p0, 0), (kb1, hp1, p1, 1),
                                    (kb0, hp0, p0, 1), (kb1, hp1, p1, 0),
                                ):
                                    emit_pv_one(kbx, hpx, px, hh)

                        for kb in range(KB):
                            run_deferred(1)
                            ks = slice(kb * 128, (kb + 1) * 128)
                            paths = [tile_path(kb, hp) for hp in range(2)]
                            s_tiles = [
                                sps.tile([128, 1024], F32, name=f"s{qc}_{kb}_{hp}", tag="s")
                                for hp in range(2)
                            ]
                            for h in range(H):
                                hs = slice(32 * h, 32 * (h + 1))
                                mm = nc.tensor.matmul(
                                    s_tiles[h // 2][:, (h % 2) * 512:(h % 2 + 1) * 512],
                                    kT_sb[hs, ks],
                                    qT_sb[hs, qs],
                                    start=True,
                                    stop=(paths[h // 2] != "P"),
                                    tile_position=(32 * h, 0),
                                )
                                last_score_mm[0] = mm.ins
                            # 'P' tiles: ride the mask add on TensorE (identity
                            # stationary, accumulate onto the score banks)
                            for hp in range(2):
                                if paths[hp] != "P":
                                    continue
                                for half in range(2):
                                    mm = nc.tensor.matmul(
                                        s_tiles[hp][:, half * 512:(half + 1) * 512],
                                        i128[:],
                                        mb_sb[kb][:, qs],
                                        start=False,
                                        stop=(half == 1),
                                    )
                                    last_score_mm[0] = mm.ins
                            drain_pv()
                            # emit both PSUM-freeing exps first, then the
                            # SBUF-only post-masks: the score PSUM tiles (3-buf
                            # pool = ~1.5 kb depth) are the scarce resource
                            mb3 = mb_sb[kb][:, qs].rearrange(
                                "p (o q) -> p o q", o=1
                            ).broadcast_to([128, 2, 512])
                            postmasks = []
                            for hp in range(2):
                                s_ps = s_tiles[hp]
                                p_sb = ppool.tile(
                                    [128, 1024], BF16, name=f"p{qc}_{kb}_{hp}", tag="p"
                                )
                                path = paths[hp]
                                if path == "D":
                                    out3 = p_sb[:].bitcast(I16).rearrange(
                                        "p (t q) -> p t q", t=2
                                    )
                                    in0 = s_ps[:].rearrange("p (t q) -> p t q", t=2)
                                    nc.vector.affine_then_add(
                                        out3, in0, mb3, SCHRAUD_A, SCHRAUD_B
                                    )
                                elif path == "P":
                                    nc.scalar.activation(
                                        p_sb[:], s_ps[:], EXP, bias=nbias[:]
                                    )
                                else:
                                    nc.scalar.activation(p_sb[:], s_ps[:], EXP)
                                    postmasks.append((path, p_sb))
                                pending.append((kb, hp, p_sb))
                            for path, p_sb in postmasks:
                                p3 = p_sb[:].rearrange("p (t q) -> p t q", t=2)
                                if path == "A":
                                    nc.vector.tensor_tensor(p3, p3, mb3, MIN)
                                else:
                                    ab3 = ab_sb[kb][:, qs].rearrange(
                                        "p (o q) -> p o q", o=1
                                    ).broadcast_to([128, 2, 512])
                                    nc.gpsimd.tensor_tensor(p3, p3, ab3, MULT)
                        for args_pv in pending:
                            emit_pv(*args_pv)

                        # build this qc's epilogue as deferred pieces; the
                        # chain is split into per-512-col halves with separate
                        # tiles so each stage's half-0 overlaps the previous
                        # stage's half-1 (tile-granular dep tracking)
                        def make_epilogue(qc=qc, qs=qs, acc_ps=acc_ps):
                            acc_sb = epool.tile([128, 1024], F32, name=f"accs{qc}", tag="accs")
                            rr = [epool.tile([128, 512], F32, name=f"rr{qc}_{b}", tag=f"rr{b}")
                                  for b in range(2)]
                            rrb = [epool.tile([128, 512], BF16, name=f"rrb{qc}_{b}", tag=f"rrb{b}")
                                   for b in range(2)]
                            asc = [epool.tile([128, 512], BF16, name=f"asc{qc}_{b}", tag=f"asc{b}")
                                   for b in range(2)]
                            bcp = [None, None]
                            o_sb = epool.tile([128, 512], F32, name=f"ot{qc}", tag="ot")
                            st = {}

                            def t0():
                                nc.scalar.copy(acc_sb[:], acc_ps[:])

                            def mk_recip(b):
                                def f():
                                    nc.vector.reciprocal_approx_fast(
                                        rr[b][:], acc_sb[:, b * 512:(b + 1) * 512]
                                    )
                                    nc.gpsimd.tensor_copy(rrb[b][:], rr[b][:])
                                return f

                            def mk_bc(b):
                                def f():
                                    bcp[b] = sps.tile([128, 512], F32, name=f"bc{qc}_{b}", tag="s")
                                    nc.tensor.matmul(bcp[b][:], selb[:], rrb[b][:])
                                return f

                            def mk_asc_wp(b):
                                def f():
                                    nc.vector.scalar_tensor_tensor(
                                        asc[b][:], bcp[b][:], 1.0,
                                        acc_sb[:, b * 512:(b + 1) * 512], MULT, MULT
                                    )
                                    if b == 0:
                                        st["o2"] = sps.tile([128, 512], F32, name=f"o2_{qc}", tag="s")
                                    nc.tensor.matmul(
                                        st["o2"][:],
                                        (w_p0, w_p1)[b],
                                        asc[b][:],
                                        start=(b == 0),
                                        stop=False,
                                    )
                                    if b == 1:
                                        # residual rides the projection: o2 += x
                                        nc.tensor.matmul(
                                            st["o2"][:], i128[:], xr_sb[:, qs],
                                            start=False, stop=True,
                                        )
                                return f

                            def t6():
                                nc.vector.tensor_scalar_add(
                                    o_sb[:], st["o2"][:], bp_sb[:]
                                )
                                nc.sync.dma_start(outT[:, qs], o_sb[:])

                            return [t0, mk_recip(0), mk_recip(1), mk_bc(0),
                                    mk_bc(1), mk_asc_wp(0), mk_asc_wp(1), t6]

                        deferred.extend(make_epilogue())

                    run_deferred(len(deferred))

    nc.compile()
    return nc


import numpy as np

_CACHE = {}


def _prep_in_maps(x, A, Wq, Wkv, Wp, bp):
    import ml_dtypes

    bf16 = ml_dtypes.bfloat16
    x = np.asarray(x, np.float32)
    A = np.asarray(A)
    Wq = np.asarray(Wq, np.float32)
    Wkv = np.asarray(Wkv, np.float32)
    Wp = np.asarray(Wp, np.float32)
    bp = np.asarray(bp, np.float32)

    wq = np.ascontiguousarray((Wq * SCALE).T).astype(bf16)
    wk = np.ascontiguousarray(Wkv[:C].T).astype(bf16)
    wv = np.ascontiguousarray(Wkv[C:].T).astype(bf16)
    bpT = np.ascontiguousarray(bp.reshape(C, 1))
    Mf = np.where(A > 0, np.float32(8192.0), np.float32(0.0))
    Ab = A.astype(np.float32)

    sel = np.zeros((C, C), np.float32)
    for j in range(C):
        sel[64 * (j // 64) + 32, j] = 1.0
    eye = np.eye(C, dtype=np.float32)
    wpT = Wp.T
    wpb = []
    for b in range(2):
        w = np.zeros((C, C), np.float32)
        for r in range(C):
            d = r % 64
            if d < 32:
                w[r, :] = wpT[32 * (2 * b + r // 64) + d, :]
        wpb.append(np.ascontiguousarray(w).astype(bf16))

    in_maps = []
    for core in range(8):
        b, s = divmod(core, 2)
        sl = slice(s * NQ, (s + 1) * NQ)
        xTb = np.ascontiguousarray(x[b].T)
        xTb16 = xTb.astype(bf16)
        MbT = Mf[sl, :].T  # [N keys, NQ queries]
        AbT = Ab[sl, :].T
        ab01 = (np.concatenate(
            [AbT[kb * 128:(kb + 1) * 128, :] for kb in G_KBS], axis=0)
            if G_KBS else np.zeros((0, NQ), np.float32))
        in_maps.append(
            {
                "xT": xTb16,
                "xrT": np.ascontiguousarray(xTb16[:, sl]),
                "MB": np.ascontiguousarray(MbT).astype(bf16),
                "AB01": np.ascontiguousarray(ab01).astype(bf16),
                "WqT": wq,
                "WkT": wk,
                "WvT": wv,
                "SELB": sel.astype(bf16),
                "I128": eye.astype(bf16),
                "WpT0": wpb[0],
                "WpT1": wpb[1],
                "bpT": bpT,
            }
        )
    return in_maps


def kernel(x, A, Wq, Wkv, Wp, bp):
    from concourse.bass_utils import run_bass_kernel_spmd

    if "nc" not in _CACHE:
        _CACHE["nc"] = _build()
    nc = _CACHE["nc"]
    in_maps = _prep_in_maps(x, A, Wq, Wkv, Wp, bp)
    res = run_bass_kernel_spmd(nc, in_maps, list(range(8)))
    out = np.empty((B, N, C), np.float32)
    for core in range(8):
        b, s = divmod(core, 2)
        out[b, s * NQ:(s + 1) * NQ, :] = res.results[core]["outT"].T
    return out
